# revision 1
# baseline (speedup 1.0000x reference)
"""FDSCS front-end as two Bass/Tile kernels on 8 Trainium2 NeuronCores.

Kernel A (row-sharded, 8 cores x 48 half-res rows): avg-pool 2x2 -> YCbCr ->
5x5 census on Y emitted as 24 bf16 bit-planes; cb/cr planes pre-scaled by the
unify constants. The Hamming cost volume runs on the (otherwise idle) tensor
engine: ham(x, u) = popL(u) + popR(x)*[u<W] - 2*sum_k Lbit_k(u) Rbit_k(x) is
ONE 72-row bit-plane matmul per (row, x-block) -- popcounts and the right-edge
mask are folded in as extra contraction rows, so no SWAR popcount and no DVE
masking exist at all. Band-diagonal psum tiles [x, u-window] are normalized
((h-MY)/SY) to f16 by Act/DVE and stored as scratch; the host extracts the
d = u - x band with a zero-copy as_strided view during unsharding.

Kernel B (disparity-sharded, cyclic d = 8*dp + core): Cb/Cr cost volumes as
f16 abs-diffs with operands read in-place via AP column offsets (no operand
DMA). The three row-group mask strips are applied in a single 3D op; the
normalize-and-cast runs on Act; the wire format is f16 (host widens to f32).
"""

import numpy as np

# ---------------------------------------------------------------- constants
N, HF, WF = 2, 384, 1280       # full-res input (per image): (N, 3, HF, WF)
H, W = 192, 640                # half-res
D = 128                        # disparities
NC = 8                         # cores
RPC = H * N // NC              # 48 half-rows per kernel-A core
UW = 768                       # left (u) extent incl. disparity pad
NB = 5                         # x-blocks of 128
PITCH = 768                    # staged plane pitch for kernel B
LW = 760                       # kernel-B left-plane width  (W + 15*8)
NDP = 16                       # disparities per kernel-B core (d = 8*dp + c)
NH = N * H                     # 384 half-rows total

MY, SY = 11.08282948, 0.1949711
MU, SU = 0.02175535, 35.91432953
MV, SV = 0.02679042, 26.79782867

OFFSETS = [(0,0),(1,0),(2,0),(3,0),(4,0),(0,1),(1,1),(2,1),(3,1),(4,1),
           (0,2),(1,2),(3,2),(4,2),(0,3),(1,3),(2,3),(3,3),(4,3),
           (0,4),(1,4),(2,4),(3,4),(4,4)]

_CACHE = {}


def _bass_mods():
    import concourse.bass as bass
    import concourse.tile as tile
    from concourse import bacc, mybir
    return bass, tile, bacc, mybir


def _ts_i(eng, mybir, out, in0, s1, s2, op0, op1, imm_dtype):
    """tensor_scalar with integer-typed immediates (bitvec ops)."""
    ins = [eng.lower_ap(in0), mybir.ImmediateValue(dtype=imm_dtype, value=s1)]
    kwargs = {}
    if s2 is not None:
        ins.append(mybir.ImmediateValue(dtype=imm_dtype, value=s2))
        kwargs["op1"] = op1
    return eng.add_instruction(
        mybir.InstTensorScalarPtr(
            name=eng.bass.get_next_instruction_name(),
            op0=op0, ins=ins, outs=[eng.lower_ap(out)], **kwargs,
        ))


# ================================================================ kernel A
def _build_A():
    bass, tile, bacc, mybir = _bass_mods()
    from concourse._compat import with_exitstack
    from contextlib import ExitStack
    dt = mybir.dt
    Alu = mybir.AluOpType
    ActF = mybir.ActivationFunctionType

    nc = bacc.Bacc("TRN2", target_bir_lowering=False, debug=False, num_devices=NC)
    rawL = nc.dram_tensor("rawL", (3, 104, WF), dt.float32, kind="ExternalInput").ap()
    rawR = nc.dram_tensor("rawR", (3, 104, WF), dt.float32, kind="ExternalInput").ap()
    rowmask = nc.dram_tensor("rowmask", (104, 4), dt.float32, kind="ExternalInput").ap()
    scr = nc.dram_tensor("scr", (RPC, NB, 128, 256), dt.float16, kind="ExternalOutput").ap()
    # 32 k-rows, written only in [0:24]; ExternalOutput DRAM is pre-zeroed, so
    # the pad rows/cols of the matmul planes load as zeros.
    stgL = nc.dram_tensor("stgL", (RPC, 32, UW), dt.bfloat16, kind="ExternalOutput").ap()
    stgA = nc.dram_tensor("stgA", (RPC, 32, W), dt.bfloat16, kind="ExternalOutput").ap()
    stgB = nc.dram_tensor("stgB", (RPC, 32, W), dt.bfloat16, kind="ExternalOutput").ap()
    mvconst = nc.dram_tensor("mvconst", (UW,), dt.bfloat16, kind="ExternalInput").ap()
    outs = {}
    for nm in ("lcb", "lcr", "rcb", "rcr"):
        outs[nm] = nc.dram_tensor(nm, (RPC, W), dt.float16, kind="ExternalOutput").ap()

    @with_exitstack
    def k(ctx: ExitStack, tc):
        vec, gp, act, sy = nc.vector, nc.gpsimd, nc.scalar, nc.sync
        P = 104  # 2 imgs x 52 local half-rows (48 + 4 halo)
        prep_ctx = ExitStack()
        pool = prep_ctx.enter_context(tc.tile_pool(name="prep", bufs=1))

        raw = pool.tile([P, 3 * 2 * WF], dt.float32, name="raw")
        rv = raw[:].rearrange("p (c j x) -> p c j x", c=3, j=2)
        for blk, src in ((0, rawL), (52, rawR)):
            sy.dma_start(rv[blk:blk + 52],
                         src.rearrange("c (p j) x -> p c j x", j=2))
        rm = pool.tile([P, 4], dt.float32, name="rm")
        sy.dma_start(rm[:], rowmask)

        # pooling: horizontal pair sum, vertical pair sum, x0.25 (exact XLA order)
        h = pool.tile([P, 3 * 2 * W], dt.float32, name="h")
        hv = h[:].rearrange("p (c j x) -> p c j x", c=3, j=2)
        vec.tensor_tensor(out=hv, in0=rv[:, :, :, 0::2], in1=rv[:, :, :, 1::2], op=Alu.add)
        s = pool.tile([P, 3 * W], dt.float32, name="s")
        svw = s[:].rearrange("p (c x) -> p c x", c=3)
        vec.tensor_tensor(out=svw, in0=hv[:, :, 0], in1=hv[:, :, 1], op=Alu.add)
        pooled = pool.tile([P, 3 * W], dt.float32, name="pooled")
        pv = pooled[:].rearrange("p (c x) -> p c x", c=3)
        act.activation(pooled[:], s[:], ActF.Copy, bias=0.0, scale=0.25)

        # Y = (r*.299 + g*.587) + b*.114   (channel order matches XLA assoc)
        t1 = pool.tile([P, W], dt.float32, name="t1")
        act.activation(t1[:], pv[:, 0], ActF.Copy, bias=0.0, scale=0.299)
        t2 = pool.tile([P, W], dt.float32, name="t2")
        vec.tensor_scalar(t2[:], pv[:, 1], 0.587, None, Alu.mult)
        y01 = pool.tile([P, W], dt.float32, name="y01")
        vec.tensor_tensor(out=y01[:], in0=t1[:], in1=t2[:], op=Alu.add)
        t3 = pool.tile([P, W], dt.float32, name="t3")
        act.activation(t3[:], pv[:, 2], ActF.Copy, bias=0.0, scale=0.114)
        Y = pool.tile([P, W], dt.float32, name="Y")
        vec.tensor_tensor(out=Y[:], in0=y01[:], in1=t3[:], op=Alu.add)

        # cb = ((b - y) * 0.564 + 0.5)/SU ; cr = ((r - y) * 0.713 + 0.5)/SV
        cbd = pool.tile([P, W], dt.float32, name="cbd")
        vec.scalar_tensor_tensor(cbd[:], Y[:], -1.0, pv[:, 2], Alu.mult, Alu.add)
        cb16 = pool.tile([P, W], dt.float16, name="cb16")
        act.activation(cb16[:], cbd[:], ActF.Copy, bias=0.5 / SU, scale=0.564 / SU)
        crd = pool.tile([P, W], dt.float32, name="crd")
        vec.scalar_tensor_tensor(crd[:], Y[:], -1.0, pv[:, 0], Alu.mult, Alu.add)
        cr16 = pool.tile([P, W], dt.float16, name="cr16")
        act.activation(cr16[:], crd[:], ActF.Copy, bias=0.5 / SV, scale=0.713 / SV)
        for nm, t, blk in [("lcb", cb16, 0), ("lcr", cr16, 0),
                           ("rcb", cb16, 52), ("rcr", cr16, 52)]:
            sy.dma_start(outs[nm], t[blk + 2:blk + 50, :])

        # partition-shifted copies of Y for census row offsets
        ysh = {}
        for dv in (-2, -1, 1, 2):
            t = pool.tile([P, W], dt.float32, name=f"ysh{dv + 2}")
            vec.memset(t[:], 0.0)
            for blk in (0, 52):
                if dv > 0:
                    sy.dma_start(t[blk:blk + 52 - dv], Y[blk + dv:blk + 52])
                else:
                    sy.dma_start(t[blk - dv:blk + 52], Y[blk:blk + 52 + dv])
            ysh[dv] = t
        ysh[0] = Y

        # census bits as bf16 planes [P, 24, W]; 2px x-borders stay zero
        WI = W - 4
        bits = pool.tile([P, 24 * W], dt.bfloat16, name="bits")
        bv = bits[:].rearrange("p (k x) -> p k x", k=24)
        vec.memset(bv[:, :, 0:2], 0.0)
        vec.memset(bv[:, :, W - 2:W], 0.0)
        for k_i, (u, v) in enumerate(OFFSETS):
            src = ysh[v - 2]
            vec.tensor_tensor(out=bv[:, k_i, 2:W - 2], in0=src[:, u:u + WI],
                              in1=Y[:, 2:W - 2], op=Alu.is_ge)
        # Two per-partition affine passes build all three staged operand forms:
        #   pass 1: L rows -> Lbit*mask ; R rows -> 1 - 2*Rbit       (block A)
        #   pass 2: L rows unchanged   ; R rows -> -2*Rbit*mask      (block B)
        # Stores are sliced by row group so the first matmuls start early.
        GQ = 12
        bkv = bits[:].rearrange("p (k x) -> p k x", k=24)
        vec.tensor_scalar(bits[:], bits[:], rm[:, 0:1], rm[:, 1:2], Alu.mult, Alu.add)
        for g0 in range(0, RPC, GQ):
            sy.dma_start(stgL[g0:g0 + GQ, 0:24, 0:W], bkv[2 + g0:2 + g0 + GQ])
            sy.dma_start(stgA[g0:g0 + GQ, 0:24, :], bkv[54 + g0:54 + g0 + GQ])
        vec.tensor_scalar(bits[:], bits[:], rm[:, 2:3], rm[:, 3:4], Alu.mult, Alu.add)
        for g0 in range(0, RPC, GQ):
            sy.dma_start(stgB[g0:g0 + GQ, 0:24, :], bkv[54 + g0:54 + g0 + GQ])
        prep_ctx.close()  # free all prep tiles before the plane pool opens

        # matmul operand planes: 64 contraction rows in two 32-row blocks
        # (pad rows/cols come pre-zeroed from the staging DRAM; the -0.5 const
        # block is DMA-broadcast, so no big engine memsets exist):
        #   block A 0:32   st 1-2*Rbit|0        mv  Lbit*mask|0
        #   block B 32:64  st -2*Rbit*mask|0    mv  -0.5*[u<W]
        # => psum = popL - 2*corr + popR*[u<W]  (exact integers)
        mplane = ctx.enter_context(tc.tile_pool(name="mplane", bufs=1))
        st = mplane.tile([64, RPC * W], dt.bfloat16, name="stp")
        mv = mplane.tile([64, RPC * UW], dt.bfloat16, name="mvp")
        stv = st[:].rearrange("p (q x) -> p q x", q=RPC)
        mvv = mv[:].rearrange("p (q u) -> p q u", q=RPC)
        for g0 in range(0, RPC, GQ):
            sl = bass.AP(stgL.tensor, g0 * 32 * UW, [[UW, 32], [32 * UW, GQ], [1, UW]])
            sa = bass.AP(stgA.tensor, g0 * 32 * W, [[W, 32], [32 * W, GQ], [1, W]])
            sb = bass.AP(stgB.tensor, g0 * 32 * W, [[W, 32], [32 * W, GQ], [1, W]])
            sy.dma_start(mvv[0:32, g0:g0 + GQ], sl)
            sy.dma_start(stv[0:32, g0:g0 + GQ], sa)
            sy.dma_start(stv[32:64, g0:g0 + GQ], sb)
            sy.dma_start(mvv[32:64, g0:g0 + GQ],
                         bass.AP(mvconst.tensor, 0, [[0, 32], [0, GQ], [1, UW]]))

        ppool = ctx.enter_context(tc.tile_pool(name="ps", bufs=2, space="PSUM"))
        opool = ctx.enter_context(tc.tile_pool(name="ob", bufs=4))
        for r in range(RPC):
            # one psum bank per tile; norm chunks fire as soon as their 1-2
            # matmuls land instead of waiting for the whole row
            psA = ppool.tile([128, 512], dt.float32, name="psA")
            psB = ppool.tile([128, 512], dt.float32, name="psB")
            psC = ppool.tile([128, 256], dt.float32, name="psC")
            chunks = [(psA, 0), (psA, 1), (psB, 2), (psB, 3), (psC, 4)]
            for ps, b in chunks:
                nc.tensor.matmul(
                    ps[:, (b % 2) * 256:(b % 2) * 256 + 256] if ps is not psC
                    else ps[:],
                    stv[:, r, b * 128:(b + 1) * 128],
                    mvv[:, r, b * 128:b * 128 + 256],
                    start=True, stop=True,
                )
            o = opool.tile([128, NB * 256], dt.float16, name="o")
            act.activation(o[:, 0:512], psA[:], ActF.Copy,
                           bias=-MY / SY, scale=1.0 / SY)
            vec.tensor_scalar(o[:, 512:1024], psB[:], 1.0 / SY, -MY / SY,
                              Alu.mult, Alu.add)
            if r % 2 == 0:
                act.activation(o[:, 1024:1280], psC[:], ActF.Copy,
                               bias=-MY / SY, scale=1.0 / SY)
            else:
                vec.tensor_scalar(o[:, 1024:1280], psC[:], 1.0 / SY, -MY / SY,
                                  Alu.mult, Alu.add)
            sy.dma_start(scr[r].rearrange("b x u -> x b u"),
                         o[:].rearrange("p (b u) -> p b u", b=NB))

    with tile.TileContext(nc) as tc:
        k(tc)
    nc.compile()
    return nc


# ================================================================ kernel B
def _build_B():
    bass, tile, bacc, mybir = _bass_mods()
    from concourse._compat import with_exitstack
    from contextlib import ExitStack
    dt = mybir.dt
    Alu = mybir.AluOpType
    ActF = mybir.ActivationFunctionType

    nc = bacc.Bacc("TRN2", target_bir_lowering=False, debug=False, num_devices=NC)
    ins = {}
    for nm, wid in [("Lcb", LW), ("Lcr", LW), ("Rcb", W), ("Rcr", W)]:
        ins[nm] = nc.dram_tensor(nm, (NH, wid), dt.float16, kind="ExternalInput").ap()
    out = nc.dram_tensor("out", (2, NDP, NH, W), dt.float16, kind="ExternalOutput").ap()

    RG = NH // 128  # 3 row groups

    @with_exitstack
    def k(ctx: ExitStack, tc):
        vec, gp, act, sy = nc.vector, nc.gpsimd, nc.scalar, nc.sync
        F = RG * W  # 1920

        plane_pool = ctx.enter_context(tc.tile_pool(name="planes", bufs=1))
        planes = {}
        for nm in ("Lcb", "Lcr", "Rcb", "Rcr"):
            wid = LW if nm.startswith("L") else W
            t = plane_pool.tile([128, RG * wid], dt.float16, name=f"pl_{nm}")
            sy.dma_start(t[:].rearrange("p (g x) -> p g x", g=RG),
                         ins[nm].rearrange("(g p) x -> p g x", p=128))
            planes[nm] = t

        dpool = ctx.enter_context(tc.tile_pool(name="dp", bufs=4))
        fpool = ctx.enter_context(tc.tile_pool(name="fp", bufs=4))

        def Lv(nm, off):
            return planes[nm][:].rearrange("p (g x) -> p g x", g=RG)[:, :, off:off + W]

        def Rv(nm):
            return planes[nm][:].rearrange("p (g x) -> p g x", g=RG)

        # right-edge triangle (x >= W - d) is NOT masked on-device: the host
        # overwrites it with the constant -M/S during unshard.
        for dp in range(NDP):
            off = 8 * dp
            for gi, (lnm, rnm, bias) in enumerate(
                    [("Lcb", "Rcb", -MU / SU), ("Lcr", "Rcr", -MV / SV)]):
                i = 2 * dp + gi
                du = dpool.tile([128, F], dt.float16, name="du")
                eng = gp if i % 3 == 2 else vec
                eng.tensor_tensor(out=du[:].rearrange("p (g x) -> p g x", g=RG),
                                  in0=Lv(lnm, off), in1=Rv(rnm), op=Alu.subtract)
                cF = fpool.tile([128, F], dt.float16, name="cF")
                if i % 5 < 2:
                    # Act takes |du| (Abs activation), DVE the affine (4x mode)
                    ab = dpool.tile([128, F], dt.float16, name="ab")
                    act.activation(ab[:], du[:], ActF.Abs, bias=0.0, scale=1.0)
                    vec.tensor_scalar(cF[:], ab[:], 1.0, bias, Alu.mult, Alu.add)
                else:
                    ab = dpool.tile([128, F], dt.float16, name="ab")
                    _ts_i(vec, mybir, ab[:].bitcast(dt.uint16),
                          du[:].bitcast(dt.uint16),
                          0x7FFF, None, Alu.bitwise_and, None, dt.uint16)
                    if i % 5 < 4:
                        act.activation(cF[:], ab[:], ActF.Copy, bias=bias, scale=1.0)
                    else:
                        vec.tensor_scalar(cF[:], ab[:], 1.0, bias, Alu.mult, Alu.add)
                sy.dma_start(out[gi, dp].rearrange("(g p) x -> p g x", p=128),
                             cF[:].rearrange("p (g x) -> p g x", g=RG))

    with tile.TileContext(nc) as tc:
        k(tc)
    nc.compile()
    return nc


# ================================================================ host
def _run(nc, in_maps):
    from concourse.bass_utils import run_bass_kernel_spmd
    return run_bass_kernel_spmd(nc, in_maps, core_ids=list(range(NC)))


def kernel(left, right):
    import ml_dtypes
    left = np.asarray(left, dtype=np.float32)
    right = np.asarray(right, dtype=np.float32)

    if "A" not in _CACHE:
        _CACHE["A"] = _build_A()
    if "B" not in _CACHE:
        _CACHE["B"] = _build_B()

    # ---------------- kernel A launch (row-sharded)
    in_mapsA = []
    for c in range(NC):
        n, r0 = c // 4, 48 * (c % 4)
        lo, hi = 2 * r0 - 4, 2 * (r0 + RPC) + 4
        slL = np.zeros((3, 104, WF), np.float32)
        slR = np.zeros((3, 104, WF), np.float32)
        clo, chi = max(lo, 0), min(hi, HF)
        slL[:, clo - lo:104 - (hi - chi)] = left[n, :, clo:chi]
        slR[:, clo - lo:104 - (hi - chi)] = right[n, :, clo:chi]
        # per-partition affine coefficients for the two staging passes:
        #   pass 1: bits*c0 + c1   -> L: Lb*mask, R: 1-2*Rb
        #   pass 2: bits*c2 + c3   -> L: unchanged, R: -2*Rb*mask = (cur-1)*mask
        rmv = np.zeros((104, 4), np.float32)
        for i in range(48):
            m = 0.0 if (r0 + i) in (0, 1, H - 2, H - 1) else 1.0
            rmv[2 + i] = (m, 0.0, 1.0, 0.0)
            rmv[54 + i] = (-2.0, 1.0, m, -m)
        mvc = np.zeros((UW,), np.float32)
        mvc[:W] = -0.5
        in_mapsA.append({"rawL": slL, "rawR": slR, "rowmask": rmv,
                         "mvconst": mvc.astype(ml_dtypes.bfloat16)})
    resA = _run(_CACHE["A"], in_mapsA)

    # ---------------- assemble staged canvases for kernel B
    canv = {nm: np.zeros((NH, PITCH), np.float16)
            for nm in ("lcb", "lcr", "rcb", "rcr")}
    for c in range(NC):
        for nm in canv:
            canv[nm][48 * c:48 * (c + 1), :W] = resA.results[c][nm]

    # ---------------- kernel B launch (disparity-sharded)
    in_mapsB = []
    for c in range(NC):
        m = {
            "Lcb": np.ascontiguousarray(canv["lcb"][:, c:c + LW]),
            "Lcr": np.ascontiguousarray(canv["lcr"][:, c:c + LW]),
            "Rcb": np.ascontiguousarray(canv["rcb"][:, :W]),
            "Rcr": np.ascontiguousarray(canv["rcr"][:, :W]),
        }
        in_mapsB.append(m)
    resB = _run(_CACHE["B"], in_mapsB)

    # ---------------- assemble output
    outf = np.empty((N, 3 * D, H, W), np.float32)
    # y-group: de-shear the band scratch (d = u - x) during unshard
    scr = np.empty((NH, NB, 128, 256), np.float16)
    for c in range(NC):
        scr[48 * c:48 * (c + 1)] = resA.results[c]["scr"]
    s = scr.strides
    v = np.lib.stride_tricks.as_strided(
        scr, shape=(D, NH, NB, 128),
        strides=(s[3], s[0], s[1], s[2] + s[3]))
    vv = v.reshape(D, N, H, NB * 128)
    for n in range(N):
        outf[n, 0:D] = vv[:, n]
    # u/v groups from kernel B; the x >= W-d triangle is filled with the
    # normalized-zero constant during unshard (not masked on device)
    for c in range(NC):
        o = resB.results[c]["out"]          # (2, NDP, NH, W) f16
        ov = o.reshape(2, NDP, N, H, W)
        for gi in range(2):
            for dp in range(NDP):
                outf[:, (1 + gi) * D + 8 * dp + c] = ov[gi, dp]
    for gi, cst in ((1, -MU / SU), (2, -MV / SV)):
        for d in range(1, D):
            outf[:, gi * D + d, :, W - d:] = np.float32(cst)
    return outf



# revision 13
# speedup vs baseline: 1.0653x; 1.0653x over previous
"""FDSCS front-end as two Bass/Tile kernels on 8 Trainium2 NeuronCores.

Kernel A (row-sharded, 8 cores x 48 half-res rows): avg-pool 2x2 -> YCbCr ->
5x5 census on Y emitted as fp8 bit-planes. The Hamming cost volume runs on the
tensor engine as ONE 24-row fp8 matmul per (row, x-block):
  st = 1-2*Rbit in {-1,+1},  mv = Lbit-0.5*[u<W] in {-0.5,+0.5} (0 for u>=W)
  psum = popL - 2*corr + (popR-12)*[u<W] = ham - 12   (exact integers)
Act/DVE/Pool add +12 and cast to uint8; scr stores the raw 2x band as u8
(1152B contiguous rows). The host de-shears the d = u - x band with a
zero-copy as_strided view and applies (ham-MY)/SY during unshard; border rows
and the x >= W-d triangle are constant host fills (no masks exist on device).

Kernel B (disparity-sharded, cyclic d = 8*dp + core): Cb/Cr cost volumes as
SIGNED f8 diffs (planes pre-scaled by the unify constants in kernel A);
engines rotate over {DVE sub->fp8, DVE sub->f16 + Act cast, Pool sub->fp8}.
The host takes |.|, applies the -M/S bias, and fills the right-edge triangle.
"""

import numpy as np

# ---------------------------------------------------------------- constants
N, HF, WF = 2, 384, 1280       # full-res input (per image): (N, 3, HF, WF)
H, W = 192, 640                # half-res
D = 128                        # disparities
NC = 8                         # cores
RPC = H * N // NC              # 48 half-rows per kernel-A core
UW = 768                       # left (u) extent incl. disparity pad
SRW = 1152                     # scr row width: 4 x 256 + 128 (b=4 trimmed)
PITCH = 768                    # staged plane pitch for kernel B
LW = 760                       # kernel-B left-plane width  (W + 15*8)
NDP = 16                       # disparities per kernel-B core (d = 8*dp + c)
NH = N * H                     # 384 half-rows total
GQ = 12                        # staging row-group quantum

MY, SY = 11.08282948, 0.1949711
MU, SU = 0.02175535, 35.91432953
MV, SV = 0.02679042, 26.79782867

OFFSETS = [(0,0),(1,0),(2,0),(3,0),(4,0),(0,1),(1,1),(2,1),(3,1),(4,1),
           (0,2),(1,2),(3,2),(4,2),(0,3),(1,3),(2,3),(3,3),(4,3),
           (0,4),(1,4),(2,4),(3,4),(4,4)]

# partition layout for kernel A: [2 pad][52 L rows][4 pad][52 R rows][2 pad]
P = 112
LB, RB = 2, 58                 # base partitions of the L / R row blocks

_CACHE = {}


def _bass_mods():
    import concourse.bass as bass
    import concourse.tile as tile
    from concourse import bacc, mybir
    return bass, tile, bacc, mybir


# ================================================================ kernel A
def _build_A():
    bass, tile, bacc, mybir = _bass_mods()
    from concourse._compat import with_exitstack
    from contextlib import ExitStack
    dt = mybir.dt
    Alu = mybir.AluOpType
    ActF = mybir.ActivationFunctionType

    nc = bacc.Bacc("TRN2", target_bir_lowering=False, debug=False, num_devices=NC)
    # 112 full-res rows each: pad rows come in as host zeros so every
    # partition of the raw tile is written (L block: partitions 0:56,
    # R block: 56:112; data at half-rows 2:54 of each block).
    rawL = nc.dram_tensor("rawL", (3, P, WF), dt.float32, kind="ExternalInput").ap()
    rawR = nc.dram_tensor("rawR", (3, P, WF), dt.float32, kind="ExternalInput").ap()
    rmt = nc.dram_tensor("rmt", (P, 2), dt.float32, kind="ExternalInput").ap()
    scr = nc.dram_tensor("scr", (RPC, 128, SRW), dt.uint8, kind="ExternalOutput").ap()
    # staged census planes; ExternalOutput DRAM is pre-zeroed, so stgL's
    # cols [W:UW] read back as zeros (the disparity pad of the mv operand).
    stgL = nc.dram_tensor("stgL", (RPC, 24, UW), dt.float8e4, kind="ExternalOutput").ap()
    stgR = nc.dram_tensor("stgR", (RPC, 24, W), dt.float8e4, kind="ExternalOutput").ap()
    outs = {}
    for nm in ("lcb", "lcr", "rcb", "rcr"):
        outs[nm] = nc.dram_tensor(nm, (RPC, W), dt.float16, kind="ExternalOutput").ap()

    @with_exitstack
    def k(ctx: ExitStack, tc):
        vec, gp, act, sy = nc.vector, nc.gpsimd, nc.scalar, nc.sync
        prep_ctx = ExitStack()
        pool = prep_ctx.enter_context(tc.tile_pool(name="prep", bufs=1))

        raw = pool.tile([P, 3 * 2 * WF], dt.float32, name="raw")
        rv = raw[:].rearrange("p (c j x) -> p c j x", c=3, j=2)
        for blk, src in ((0, rawL), (56, rawR)):
            sy.dma_start(rv[blk:blk + 56],
                         src.rearrange("c (p j) x -> p c j x", j=2))
        rm = pool.tile([P, 2], dt.float32, name="rm")
        sy.dma_start(rm[:], rmt)

        # pooling: horizontal pair sum, vertical pair sum, x0.25 (exact XLA order)
        h = pool.tile([P, 3 * 2 * W], dt.float32, name="h")
        hv = h[:].rearrange("p (c j x) -> p c j x", c=3, j=2)
        gp.tensor_tensor(out=hv, in0=rv[:, :, :, 0::2], in1=rv[:, :, :, 1::2], op=Alu.add)
        s = pool.tile([P, 3 * W], dt.float32, name="s")
        svw = s[:].rearrange("p (c x) -> p c x", c=3)
        gp.tensor_tensor(out=svw, in0=hv[:, :, 0], in1=hv[:, :, 1], op=Alu.add)
        pooled = pool.tile([P, 3 * W], dt.float32, name="pooled")
        pv = pooled[:].rearrange("p (c x) -> p c x", c=3)
        act.activation(pooled[:], s[:], ActF.Copy, bias=0.0, scale=0.25)

        # Y = (r*.299 + g*.587) + b*.114   (channel order matches XLA assoc)
        t1 = pool.tile([P, W], dt.float32, name="t1")
        act.activation(t1[:], pv[:, 0], ActF.Copy, bias=0.0, scale=0.299)
        t2 = pool.tile([P, W], dt.float32, name="t2")
        vec.tensor_scalar(t2[:], pv[:, 1], 0.587, None, Alu.mult)
        y01 = pool.tile([P, W], dt.float32, name="y01")
        gp.tensor_tensor(out=y01[:], in0=t1[:], in1=t2[:], op=Alu.add)
        t3 = pool.tile([P, W], dt.float32, name="t3")
        act.activation(t3[:], pv[:, 2], ActF.Copy, bias=0.0, scale=0.114)
        Y = pool.tile([P, W], dt.float32, name="Y")
        gp.tensor_tensor(out=Y[:], in0=y01[:], in1=t3[:], op=Alu.add)

        # cb = ((b - y) * 0.564 + 0.5)/SU ; cr = ((r - y) * 0.713 + 0.5)/SV
        cbd = pool.tile([P, W], dt.float32, name="cbd")
        vec.scalar_tensor_tensor(cbd[:], Y[:], -1.0, pv[:, 2], Alu.mult, Alu.add)
        cb16 = pool.tile([P, W], dt.float16, name="cb16")
        act.activation(cb16[:], cbd[:], ActF.Copy, bias=0.5 / SU, scale=0.564 / SU)
        crd = pool.tile([P, W], dt.float32, name="crd")
        vec.scalar_tensor_tensor(crd[:], Y[:], -1.0, pv[:, 0], Alu.mult, Alu.add)
        cr16 = pool.tile([P, W], dt.float16, name="cr16")
        act.activation(cr16[:], crd[:], ActF.Copy, bias=0.5 / SV, scale=0.713 / SV)
        for nm, t, blk in [("lcb", cb16, LB), ("lcr", cr16, LB),
                           ("rcb", cb16, RB), ("rcr", cr16, RB)]:
            sy.dma_start(outs[nm], t[blk + 2:blk + 50, :])

        # Y's pad rows are computed zeros (host-zero raw pads), so the +-2
        # partition shifts pull zeros across image boundaries for free.
        # partition-shifted copies of Y: one whole-tile DMA per row offset
        ysh = {0: Y}
        for dv in (-2, -1, 1, 2):
            t = pool.tile([P, W], dt.float32, name=f"ysh{dv + 2}")
            if dv > 0:
                sy.dma_start(t[0:P - dv], Y[dv:P])
                sy.dma_start(t[P - dv:P], Y[0:dv])   # filler: rows unused
            else:
                sy.dma_start(t[-dv:P], Y[0:P + dv])
                sy.dma_start(t[0:-dv], Y[0:-dv])     # filler: rows unused
            ysh[dv] = t

        # census bits as fp8 planes [P, 24, W]; 2px x-borders stay zero.
        # engine rotation: DVE 663ns / Pool 1261ns per op — Pool takes a share
        # so DVE keeps room for the affine + its norm chunks.
        WI = W - 4
        bits = pool.tile([P, 24 * W], dt.float8e4, name="bits")
        bv = bits[:].rearrange("p (k x) -> p k x", k=24)
        vec.memset(bv[:, :, 0:2], 0.0)
        vec.memset(bv[:, :, W - 2:W], 0.0)
        for k_i, (u, v) in enumerate(OFFSETS):
            src = ysh[v - 2]
            eng = vec
            eng.tensor_tensor(out=bv[:, k_i, 2:W - 2],
                              in0=src[:, u:u + WI],
                              in1=Y[:, 2:W - 2], op=Alu.is_ge)
        # one per-partition affine pass builds both staged operand forms:
        #   L rows: bit - 0.5          R rows: 1 - 2*bit
        gp.tensor_scalar(bits[:], bits[:], rm[:, 0:1], rm[:, 1:2],
                          Alu.mult, Alu.add)
        for g0 in range(0, RPC, GQ):
            sy.dma_start(stgL[g0:g0 + GQ, :, 0:W], bv[LB + 2 + g0:LB + 2 + g0 + GQ])
            sy.dma_start(stgR[g0:g0 + GQ], bv[RB + 2 + g0:RB + 2 + g0 + GQ])
        prep_ctx.close()  # free all prep tiles before the plane pool opens

        # matmul operand planes, k-major: st [24, q, W], mv [24, q, UW]
        mplane = ctx.enter_context(tc.tile_pool(name="mplane", bufs=1))
        st = mplane.tile([24, RPC * W], dt.float8e4, name="stp")
        mv = mplane.tile([24, RPC * UW], dt.float8e4, name="mvp")
        stv = st[:].rearrange("p (q x) -> p q x", q=RPC)
        mvv = mv[:].rearrange("p (q u) -> p q u", q=RPC)
        for g0 in range(0, RPC, GQ):
            sl = bass.AP(stgL.tensor, g0 * 24 * UW, [[UW, 24], [24 * UW, GQ], [1, UW]])
            sr = bass.AP(stgR.tensor, g0 * 24 * W, [[W, 24], [24 * W, GQ], [1, W]])
            sy.dma_start(mvv[:, g0:g0 + GQ], sl)
            sy.dma_start(stv[:, g0:g0 + GQ], sr)

        ppool = ctx.enter_context(tc.tile_pool(name="ps", bufs=2, space="PSUM"))
        opool = ctx.enter_context(tc.tile_pool(name="ob", bufs=4))
        for r in range(RPC):
            psA = ppool.tile([128, 512], dt.float32, name="psA")
            psB = ppool.tile([128, 512], dt.float32, name="psB")
            psC = ppool.tile([128, 128], dt.float32, name="psC")
            for b in range(4):
                ps = psA if b < 2 else psB
                nc.tensor.matmul(
                    ps[:, (b % 2) * 256:(b % 2) * 256 + 256],
                    stv[:, r, b * 128:(b + 1) * 128],
                    mvv[:, r, b * 128:b * 128 + 256],
                    start=True, stop=True,
                )
            nc.tensor.matmul(psC[:], stv[:, r, 512:640], mvv[:, r, 512:640],
                             start=True, stop=True)
            o = opool.tile([128, SRW], dt.uint8, name="o")
            # +12 and cast to u8; c0 on Act, c2 on DVE, c1 mostly Pool
            # (weights chosen so all three engines land near 40us total)
            e1 = act if r % 2 == 0 else vec
            chunks = [(o[:, 0:512], psA), (o[:, 512:1024], psB),
                      (o[:, 1024:1152], psC)]
            for (dst, ps), eng in zip(chunks, (act, e1, vec)):
                if eng is act:
                    act.activation(dst, ps[:], ActF.Copy, bias=12.0, scale=1.0)
                else:
                    eng.tensor_scalar(dst, ps[:], 1.0, 12.0, Alu.mult, Alu.add)
            sy.dma_start(scr[r], o[:])

    with tile.TileContext(nc) as tc:
        k(tc)
    nc.compile()
    return nc


# ================================================================ kernel B
def _build_B():
    bass, tile, bacc, mybir = _bass_mods()
    from concourse._compat import with_exitstack
    from contextlib import ExitStack
    dt = mybir.dt
    Alu = mybir.AluOpType
    ActF = mybir.ActivationFunctionType

    nc = bacc.Bacc("TRN2", target_bir_lowering=False, debug=False, num_devices=NC)
    ins = {}
    for nm, wid in [("Lcb", LW), ("Lcr", LW), ("Rcb", W), ("Rcr", W)]:
        ins[nm] = nc.dram_tensor(nm, (NH, wid), dt.float16, kind="ExternalInput").ap()
    out = nc.dram_tensor("out", (2, NDP, NH, W), dt.float8e4, kind="ExternalOutput").ap()

    RG = NH // 128  # 3 row groups

    @with_exitstack
    def k(ctx: ExitStack, tc):
        vec, gp, act, sy = nc.vector, nc.gpsimd, nc.scalar, nc.sync

        plane_pool = ctx.enter_context(tc.tile_pool(name="planes", bufs=1))
        planes = {}
        for nm in ("Lcb", "Lcr", "Rcb", "Rcr"):
            wid = LW if nm.startswith("L") else W
            t = plane_pool.tile([128, RG * wid], dt.float16, name=f"pl_{nm}")
            sy.dma_start(t[:].rearrange("p (g x) -> p g x", g=RG),
                         ins[nm].rearrange("(g p) x -> p g x", p=128))
            planes[nm] = t

        dpool = ctx.enter_context(tc.tile_pool(name="dp", bufs=4))
        fpool = ctx.enter_context(tc.tile_pool(name="fp", bufs=4))

        def Lv(nm, off, wt):
            return planes[nm][:].rearrange("p (g x) -> p g x", g=RG)[:, :, off:off + wt]

        def Rv(nm, wt):
            return planes[nm][:].rearrange("p (g x) -> p g x", g=RG)[:, :, 0:wt]

        # signed diffs only; |.|, bias, and the right-edge triangle are host
        # work during unshard. x >= W-8*dp is never computed (host constant).
        # engine rotation P,A,A,D: Pool-direct 8, DVE-sub+Act-cast 16,
        # DVE-direct 8.
        PAT = ("P", "A", "A", "D")
        for dp in range(NDP):
            wt = W - 8 * dp
            for gi, lnm, rnm in ((0, "Lcb", "Rcb"), (1, "Lcr", "Rcr")):
                i = 2 * dp + gi
                kind = PAT[i % 4]
                c8 = fpool.tile([128, RG * W], dt.float8e4, name="c8")
                c8v = c8[:].rearrange("p (g x) -> p g x", g=RG)[:, :, 0:wt]
                if kind == "P":
                    gp.tensor_tensor(out=c8v, in0=Lv(lnm, 8 * dp, wt),
                                     in1=Rv(rnm, wt), op=Alu.subtract)
                elif kind == "D":
                    vec.tensor_tensor(out=c8v, in0=Lv(lnm, 8 * dp, wt),
                                      in1=Rv(rnm, wt), op=Alu.subtract)
                else:
                    du = dpool.tile([128, RG * W], dt.float16, name="du")
                    duv = du[:].rearrange("p (g x) -> p g x", g=RG)[:, :, 0:wt]
                    vec.tensor_tensor(out=duv, in0=Lv(lnm, 8 * dp, wt),
                                      in1=Rv(rnm, wt), op=Alu.subtract)
                    act.activation(c8v, duv, ActF.Copy, bias=0.0, scale=1.0)
                sy.dma_start(
                    out[gi, dp].rearrange("(g p) x -> p g x", p=128)[:, :, 0:wt],
                    c8v)

    with tile.TileContext(nc) as tc:
        k(tc)
    nc.compile()
    return nc


# ================================================================ host
def _run(nc, in_maps):
    from concourse.bass_utils import run_bass_kernel_spmd
    return run_bass_kernel_spmd(nc, in_maps, core_ids=list(range(NC)))


def kernel(left, right):
    left = np.asarray(left, dtype=np.float32)
    right = np.asarray(right, dtype=np.float32)

    if "A" not in _CACHE:
        _CACHE["A"] = _build_A()
    if "B" not in _CACHE:
        _CACHE["B"] = _build_B()

    # ---------------- kernel A launch (row-sharded)
    rmv = np.zeros((P, 2), np.float32)
    rmv[LB:LB + 52] = (1.0, -0.5)      # L rows: bit - 0.5
    rmv[RB:RB + 52] = (-2.0, 1.0)      # R rows: 1 - 2*bit
    in_mapsA = []
    for c in range(NC):
        n, r0 = c // 4, 48 * (c % 4)
        lo, hi = 2 * r0 - 4, 2 * (r0 + RPC) + 4
        slL = np.zeros((3, P, WF), np.float32)
        slR = np.zeros((3, P, WF), np.float32)
        clo, chi = max(lo, 0), min(hi, HF)
        # data occupies full-res rows [4:108] (pad rows 0:4 / 108:112 stay 0)
        slL[:, 4 + clo - lo:108 - (hi - chi)] = left[n, :, clo:chi]
        slR[:, 4 + clo - lo:108 - (hi - chi)] = right[n, :, clo:chi]
        in_mapsA.append({"rawL": slL, "rawR": slR, "rmt": rmv})
    resA = _run(_CACHE["A"], in_mapsA)

    # ---------------- assemble staged canvases for kernel B
    canv = {nm: np.zeros((NH, PITCH), np.float16)
            for nm in ("lcb", "lcr", "rcb", "rcr")}
    for c in range(NC):
        for nm in canv:
            canv[nm][48 * c:48 * (c + 1), :W] = resA.results[c][nm]

    # ---------------- kernel B launch (disparity-sharded)
    in_mapsB = []
    for c in range(NC):
        m = {
            "Lcb": np.ascontiguousarray(canv["lcb"][:, c:c + LW]),
            "Lcr": np.ascontiguousarray(canv["lcr"][:, c:c + LW]),
            "Rcb": np.ascontiguousarray(canv["rcb"][:, :W]),
            "Rcr": np.ascontiguousarray(canv["rcr"][:, :W]),
        }
        in_mapsB.append(m)
    resB = _run(_CACHE["B"], in_mapsB)

    # ---------------- assemble output
    outf = np.empty((N, 3 * D, H, W), np.float32)
    # y-group: de-shear the u8 band scratch (d = u - x) during unshard.
    # 256-byte tail pad keeps the b=4 as_strided view in-bounds.
    flat = np.zeros(NH * 128 * SRW + 256, np.uint8)
    scr = flat[:NH * 128 * SRW].reshape(NH, 128, SRW)
    for c in range(NC):
        scr[48 * c:48 * (c + 1)] = resA.results[c]["scr"]
    s0 = 128 * SRW
    vb = np.lib.stride_tricks.as_strided(
        scr, shape=(D, NH, 4, 128), strides=(1, s0, 256, SRW + 1))
    v4 = np.lib.stride_tricks.as_strided(
        scr[:, :, 1024:], shape=(D, NH, 128), strides=(1, s0, SRW + 1))
    yf = np.empty((D, NH, W), np.float32)
    yf[:, :, 0:512] = vb.reshape(D, NH, 512)
    yf[:, :, 512:W] = v4
    yf -= np.float32(MY)
    yf *= np.float32(1.0 / SY)
    for n in range(N):
        outf[n, 0:D] = yf[:, n * H:(n + 1) * H]
    # u/v groups from kernel B: |.| + bias during unshard
    for c in range(NC):
        o = np.abs(resB.results[c]["out"].astype(np.float32))
        ov = o.reshape(2, NDP, N, H, W)
        for gi, bias in ((0, MU / SU), (1, MV / SV)):
            for dp in range(NDP):
                outf[:, (1 + gi) * D + 8 * dp + c] = ov[gi, dp] - np.float32(bias)
    # constant fills: x >= W-d triangle (all groups) and census border rows (y)
    for gi, cst in ((0, -MY / SY), (1, -MU / SU), (2, -MV / SV)):
        for d in range(1, D):
            outf[:, gi * D + d, :, W - d:] = np.float32(cst)
    outf[:, 0:D, (0, 1, H - 2, H - 1), :] = np.float32(-MY / SY)
    return outf


# revision 16
# speedup vs baseline: 1.2389x; 1.1630x over previous
"""FDSCS front-end as two Bass/Tile kernels on 8 Trainium2 NeuronCores.

Kernel A (row-sharded, 8 cores x 48 half-res rows): avg-pool 2x2 -> YCbCr ->
5x5 census on Y emitted as fp8 bit-planes. The Hamming cost volume runs on the
tensor engine as ONE 24-row fp8 matmul per (row, x-block):
  st = 1-2*Rbit in {-1,+1},  mv = Lbit-0.5*[u<W] in {-0.5,+0.5} (0 for u>=W)
  psum = popL - 2*corr + (popR-12)*[u<W] = ham - 12   (exact integers)
Act/DVE/Pool add +12 and cast to uint8; scr stores the raw 2x band as u8
(1152B contiguous rows). The host de-shears the d = u - x band with a
zero-copy as_strided view and applies (ham-MY)/SY during unshard; border rows
and the x >= W-d triangle are constant host fills (no masks exist on device).

Kernel B (disparity-sharded, cyclic d = 8*dp + core): Cb/Cr cost volumes as
SIGNED f8 diffs (planes pre-scaled by the unify constants in kernel A);
engines rotate over {DVE sub->fp8, DVE sub->f16 + Act cast, Pool sub->fp8}.
The host takes |.|, applies the -M/S bias, and fills the right-edge triangle.
"""

import numpy as np

# ---------------------------------------------------------------- constants
N, HF, WF = 2, 384, 1280       # full-res input (per image): (N, 3, HF, WF)
H, W = 192, 640                # half-res
D = 128                        # disparities
NC = 8                         # cores
RPC = H * N // NC              # 48 half-rows per kernel-A core
UW = 768                       # left (u) extent incl. disparity pad
SRW = 1152                     # scr row width: 4 x 256 + 128 (b=4 trimmed)
PITCH = 768                    # staged plane pitch for kernel B
LW = 760                       # kernel-B left-plane width  (W + 15*8)
NDP = 16                       # disparities per kernel-B core (d = 8*dp + c)
NH = N * H                     # 384 half-rows total
GQ = 12                        # staging row-group quantum

MY, SY = 11.08282948, 0.1949711
MU, SU = 0.02175535, 35.91432953
MV, SV = 0.02679042, 26.79782867

OFFSETS = [(0,0),(1,0),(2,0),(3,0),(4,0),(0,1),(1,1),(2,1),(3,1),(4,1),
           (0,2),(1,2),(3,2),(4,2),(0,3),(1,3),(2,3),(3,3),(4,3),
           (0,4),(1,4),(2,4),(3,4),(4,4)]

# partition layout for kernel A: [2 pad][52 L rows][4 pad][52 R rows][2 pad]
P = 112
LB, RB = 2, 58                 # base partitions of the L / R row blocks

_CACHE = {}


def _bass_mods():
    import concourse.bass as bass
    import concourse.tile as tile
    from concourse import bacc, mybir
    return bass, tile, bacc, mybir


# ================================================================ kernel A
def _build_A():
    bass, tile, bacc, mybir = _bass_mods()
    from concourse._compat import with_exitstack
    from contextlib import ExitStack
    dt = mybir.dt
    Alu = mybir.AluOpType
    ActF = mybir.ActivationFunctionType

    nc = bacc.Bacc("TRN2", target_bir_lowering=False, debug=False, num_devices=NC)
    # 112 full-res rows each: pad rows come in as host zeros so every
    # partition of the raw tile is written (L block: partitions 0:56,
    # R block: 56:112; data at half-rows 2:54 of each block).
    rawL = nc.dram_tensor("rawL", (3, P, WF), dt.float32, kind="ExternalInput").ap()
    rawR = nc.dram_tensor("rawR", (3, P, WF), dt.float32, kind="ExternalInput").ap()
    rmt = nc.dram_tensor("rmt", (P, 2), dt.float32, kind="ExternalInput").ap()
    scr = nc.dram_tensor("scr", (RPC, 128, SRW), dt.uint8, kind="ExternalOutput").ap()
    # staged census planes; ExternalOutput DRAM is pre-zeroed, so stgL's
    # cols [W:UW] read back as zeros (the disparity pad of the mv operand).
    stgL = nc.dram_tensor("stgL", (RPC, 24, UW), dt.float8e4, kind="ExternalOutput").ap()
    stgR = nc.dram_tensor("stgR", (RPC, 24, W), dt.float8e4, kind="ExternalOutput").ap()
    outs = {}
    for nm in ("lcb", "lcr", "rcb", "rcr"):
        outs[nm] = nc.dram_tensor(nm, (RPC, W), dt.float16, kind="ExternalOutput").ap()

    @with_exitstack
    def k(ctx: ExitStack, tc):
        vec, gp, act, sy = nc.vector, nc.gpsimd, nc.scalar, nc.sync
        prep_ctx = ExitStack()
        pool = prep_ctx.enter_context(tc.tile_pool(name="prep", bufs=1))

        raw = pool.tile([P, 3 * 2 * WF], dt.float32, name="raw")
        rv = raw[:].rearrange("p (c j x) -> p c j x", c=3, j=2)
        for blk, src in ((0, rawL), (56, rawR)):
            sy.dma_start(rv[blk:blk + 56],
                         src.rearrange("c (p j) x -> p c j x", j=2))
        rm = pool.tile([P, 2], dt.float32, name="rm")
        sy.dma_start(rm[:], rmt)

        # pooling: horizontal pair sum, vertical pair sum, x0.25 (exact XLA order)
        h = pool.tile([P, 3 * 2 * W], dt.float32, name="h")
        hv = h[:].rearrange("p (c j x) -> p c j x", c=3, j=2)
        vec.tensor_tensor(out=hv, in0=rv[:, :, :, 0::2], in1=rv[:, :, :, 1::2], op=Alu.add)
        s = pool.tile([P, 3 * W], dt.float32, name="s")
        svw = s[:].rearrange("p (c x) -> p c x", c=3)
        vec.tensor_tensor(out=svw, in0=hv[:, :, 0], in1=hv[:, :, 1], op=Alu.add)
        pooled = pool.tile([P, 3 * W], dt.float32, name="pooled")
        pv = pooled[:].rearrange("p (c x) -> p c x", c=3)
        act.activation(pooled[:], s[:], ActF.Copy, bias=0.0, scale=0.25)

        # Y = (r*.299 + g*.587) + b*.114   (channel order matches XLA assoc)
        t1 = pool.tile([P, W], dt.float32, name="t1")
        act.activation(t1[:], pv[:, 0], ActF.Copy, bias=0.0, scale=0.299)
        t2 = pool.tile([P, W], dt.float32, name="t2")
        vec.tensor_scalar(t2[:], pv[:, 1], 0.587, None, Alu.mult)
        y01 = pool.tile([P, W], dt.float32, name="y01")
        vec.tensor_tensor(out=y01[:], in0=t1[:], in1=t2[:], op=Alu.add)
        t3 = pool.tile([P, W], dt.float32, name="t3")
        act.activation(t3[:], pv[:, 2], ActF.Copy, bias=0.0, scale=0.114)
        Y = pool.tile([P, W], dt.float32, name="Y")
        vec.tensor_tensor(out=Y[:], in0=y01[:], in1=t3[:], op=Alu.add)

        # cb = ((b - y) * 0.564 + 0.5)/SU ; cr = ((r - y) * 0.713 + 0.5)/SV
        cbd = pool.tile([P, W], dt.float32, name="cbd")
        vec.scalar_tensor_tensor(cbd[:], Y[:], -1.0, pv[:, 2], Alu.mult, Alu.add)
        cb16 = pool.tile([P, W], dt.float16, name="cb16")
        act.activation(cb16[:], cbd[:], ActF.Copy, bias=0.5 / SU, scale=0.564 / SU)
        crd = pool.tile([P, W], dt.float32, name="crd")
        vec.scalar_tensor_tensor(crd[:], Y[:], -1.0, pv[:, 0], Alu.mult, Alu.add)
        cr16 = pool.tile([P, W], dt.float16, name="cr16")
        act.activation(cr16[:], crd[:], ActF.Copy, bias=0.5 / SV, scale=0.713 / SV)
        for nm, t, blk in [("lcb", cb16, LB), ("lcr", cr16, LB),
                           ("rcb", cb16, RB), ("rcr", cr16, RB)]:
            sy.dma_start(outs[nm], t[blk + 2:blk + 50, :])

        # Y's pad rows are computed zeros (host-zero raw pads), so the +-2
        # partition shifts pull zeros across image boundaries for free.
        # partition-shifted copies of Y: one whole-tile DMA per row offset
        ysh = {0: Y}
        for dv in (-2, -1, 1, 2):
            t = pool.tile([P, W], dt.float32, name=f"ysh{dv + 2}")
            if dv > 0:
                sy.dma_start(t[0:P - dv], Y[dv:P])
                sy.dma_start(t[P - dv:P], Y[0:dv])   # filler: rows unused
            else:
                sy.dma_start(t[-dv:P], Y[0:P + dv])
                sy.dma_start(t[0:-dv], Y[0:-dv])     # filler: rows unused
            ysh[dv] = t

        # census bits as fp8 planes [P, 24, W]; 2px x-borders stay zero.
        # engine rotation: DVE 663ns / Pool 1261ns per op — Pool takes a share
        # so DVE keeps room for the affine + its norm chunks.
        WI = W - 4
        bits = pool.tile([P, 24 * W], dt.float8e4, name="bits")
        bv = bits[:].rearrange("p (k x) -> p k x", k=24)
        vec.memset(bv[:, :, 0:2], 0.0)
        vec.memset(bv[:, :, W - 2:W], 0.0)
        for k_i, (u, v) in enumerate(OFFSETS):
            src = ysh[v - 2]
            eng = vec
            eng.tensor_tensor(out=bv[:, k_i, 2:W - 2],
                              in0=src[:, u:u + WI],
                              in1=Y[:, 2:W - 2], op=Alu.is_ge)
        # one per-partition affine pass builds both staged operand forms:
        #   L rows: bit - 0.5          R rows: 1 - 2*bit
        bk = bits[:].rearrange("p (k x) -> p k x", k=24)
        vec.tensor_scalar(bk[:, 0:18], bk[:, 0:18], rm[:, 0:1], rm[:, 1:2],
                          Alu.mult, Alu.add)
        gp.tensor_scalar(bk[:, 18:24], bk[:, 18:24], rm[:, 0:1], rm[:, 1:2],
                          Alu.mult, Alu.add)
        for g0 in range(0, RPC, GQ):
            sy.dma_start(stgL[g0:g0 + GQ, :, 0:W], bv[LB + 2 + g0:LB + 2 + g0 + GQ])
            sy.dma_start(stgR[g0:g0 + GQ], bv[RB + 2 + g0:RB + 2 + g0 + GQ])
        prep_ctx.close()  # free all prep tiles before the plane pool opens

        # matmul operand planes, k-major: st [24, q, W], mv [24, q, UW]
        mplane = ctx.enter_context(tc.tile_pool(name="mplane", bufs=1))
        st = mplane.tile([24, RPC * W], dt.float8e4, name="stp")
        mv = mplane.tile([24, RPC * UW], dt.float8e4, name="mvp")
        stv = st[:].rearrange("p (q x) -> p q x", q=RPC)
        mvv = mv[:].rearrange("p (q u) -> p q u", q=RPC)
        for g0 in range(0, RPC, GQ):
            sl = bass.AP(stgL.tensor, g0 * 24 * UW, [[UW, 24], [24 * UW, GQ], [1, UW]])
            sr = bass.AP(stgR.tensor, g0 * 24 * W, [[W, 24], [24 * W, GQ], [1, W]])
            sy.dma_start(mvv[:, g0:g0 + GQ], sl)
            sy.dma_start(stv[:, g0:g0 + GQ], sr)

        ppool = ctx.enter_context(tc.tile_pool(name="ps", bufs=2, space="PSUM"))
        pcpool = ctx.enter_context(tc.tile_pool(name="psc", bufs=2, space="PSUM"))
        opool = ctx.enter_context(tc.tile_pool(name="ob", bufs=3))
        RB4 = 4  # rows per scr store
        for r0 in range(0, RPC, RB4):
            o = opool.tile([128, RB4 * SRW], dt.uint8, name="o")
            for ri in range(RB4):
                r = r0 + ri
                psA = ppool.tile([128, 512], dt.float32, name="psA")
                psB = ppool.tile([128, 512], dt.float32, name="psB")
                psC = pcpool.tile([128, 128], dt.float32, name="psC")
                for b in range(4):
                    ps = psA if b < 2 else psB
                    nc.tensor.matmul(
                        ps[:, (b % 2) * 256:(b % 2) * 256 + 256],
                        stv[:, r, b * 128:(b + 1) * 128],
                        mvv[:, r, b * 128:b * 128 + 256],
                        start=True, stop=True,
                    )
                nc.tensor.matmul(psC[:], stv[:, r, 512:640], mvv[:, r, 512:640],
                                 start=True, stop=True)
                oo = ri * SRW
                # +12 and cast to u8; c0 on Act, c1 on DVE, c2 alternating
                e2 = act if r % 2 == 0 else vec
                chunks = [(o[:, oo:oo + 512], psA, act),
                          (o[:, oo + 512:oo + 1024], psB, vec),
                          (o[:, oo + 1024:oo + 1152], psC, e2)]
                for dst, ps, eng in chunks:
                    if eng is act:
                        act.activation(dst, ps[:], ActF.Copy, bias=12.0, scale=1.0)
                    else:
                        eng.tensor_scalar(dst, ps[:], 1.0, 12.0, Alu.mult, Alu.add)
            gp.dma_start(scr[r0:r0 + RB4].rearrange("r p u -> p r u"),
                         o[:].rearrange("p (r u) -> p r u", r=RB4))

    with tile.TileContext(nc) as tc:
        k(tc)
    nc.compile()
    return nc


# ================================================================ kernel B
def _build_B():
    bass, tile, bacc, mybir = _bass_mods()
    from concourse._compat import with_exitstack
    from contextlib import ExitStack
    dt = mybir.dt
    Alu = mybir.AluOpType
    ActF = mybir.ActivationFunctionType

    nc = bacc.Bacc("TRN2", target_bir_lowering=False, debug=False, num_devices=NC)
    ins = {}
    for nm, wid in [("Lcb", LW), ("Lcr", LW), ("Rcb", W), ("Rcr", W)]:
        ins[nm] = nc.dram_tensor(nm, (NH, wid), dt.float16, kind="ExternalInput").ap()
    out = nc.dram_tensor("out", (2, NDP, NH, W), dt.float8e4, kind="ExternalOutput").ap()

    RG = NH // 128  # 3 row groups

    @with_exitstack
    def k(ctx: ExitStack, tc):
        vec, gp, act, sy = nc.vector, nc.gpsimd, nc.scalar, nc.sync

        plane_pool = ctx.enter_context(tc.tile_pool(name="planes", bufs=1))
        planes = {}
        for nm in ("Lcb", "Lcr", "Rcb", "Rcr"):
            wid = LW if nm.startswith("L") else W
            t = plane_pool.tile([128, RG * wid], dt.float16, name=f"pl_{nm}")
            sy.dma_start(t[:].rearrange("p (g x) -> p g x", g=RG),
                         ins[nm].rearrange("(g p) x -> p g x", p=128))
            planes[nm] = t

        dpool = ctx.enter_context(tc.tile_pool(name="dp", bufs=4))
        fpool = ctx.enter_context(tc.tile_pool(name="fp", bufs=4))

        def Lv(nm, off, wt):
            return planes[nm][:].rearrange("p (g x) -> p g x", g=RG)[:, :, off:off + wt]

        def Rv(nm, wt):
            return planes[nm][:].rearrange("p (g x) -> p g x", g=RG)[:, :, 0:wt]

        # signed diffs only; |.|, bias, and the right-edge triangle are host
        # work during unshard. x >= W-8*dp is never computed (host constant).
        # engine rotation P,A,A,D: Pool-direct 8, DVE-sub+Act-cast 16,
        # DVE-direct 8.
        PAT = ("P", "A", "A", "D")
        for dp in range(NDP):
            wt = W - 8 * dp
            for gi, lnm, rnm in ((0, "Lcb", "Rcb"), (1, "Lcr", "Rcr")):
                i = 2 * dp + gi
                kind = PAT[i % 4]
                c8 = fpool.tile([128, RG * W], dt.float8e4, name="c8")
                c8v = c8[:].rearrange("p (g x) -> p g x", g=RG)[:, :, 0:wt]
                if kind == "P":
                    gp.tensor_tensor(out=c8v, in0=Lv(lnm, 8 * dp, wt),
                                     in1=Rv(rnm, wt), op=Alu.subtract)
                    steng = gp
                elif kind == "D":
                    vec.tensor_tensor(out=c8v, in0=Lv(lnm, 8 * dp, wt),
                                      in1=Rv(rnm, wt), op=Alu.subtract)
                    steng = act
                else:
                    du = dpool.tile([128, RG * W], dt.float16, name="du")
                    duv = du[:].rearrange("p (g x) -> p g x", g=RG)[:, :, 0:wt]
                    vec.tensor_tensor(out=duv, in0=Lv(lnm, 8 * dp, wt),
                                      in1=Rv(rnm, wt), op=Alu.subtract)
                    act.activation(c8v, duv, ActF.Copy, bias=0.0, scale=1.0)
                    steng = act
                steng.dma_start(
                    out[gi, dp].rearrange("(g p) x -> p g x", p=128)[:, :, 0:wt],
                    c8v)

    with tile.TileContext(nc) as tc:
        k(tc)
    nc.compile()
    return nc


# ================================================================ host
def _run(nc, in_maps):
    from concourse.bass_utils import run_bass_kernel_spmd
    return run_bass_kernel_spmd(nc, in_maps, core_ids=list(range(NC)))


def kernel(left, right):
    left = np.asarray(left, dtype=np.float32)
    right = np.asarray(right, dtype=np.float32)

    if "A" not in _CACHE:
        _CACHE["A"] = _build_A()
    if "B" not in _CACHE:
        _CACHE["B"] = _build_B()

    # ---------------- kernel A launch (row-sharded)
    rmv = np.zeros((P, 2), np.float32)
    rmv[LB:LB + 52] = (1.0, -0.5)      # L rows: bit - 0.5
    rmv[RB:RB + 52] = (-2.0, 1.0)      # R rows: 1 - 2*bit
    in_mapsA = []
    for c in range(NC):
        n, r0 = c // 4, 48 * (c % 4)
        lo, hi = 2 * r0 - 4, 2 * (r0 + RPC) + 4
        slL = np.zeros((3, P, WF), np.float32)
        slR = np.zeros((3, P, WF), np.float32)
        clo, chi = max(lo, 0), min(hi, HF)
        # data occupies full-res rows [4:108] (pad rows 0:4 / 108:112 stay 0)
        slL[:, 4 + clo - lo:108 - (hi - chi)] = left[n, :, clo:chi]
        slR[:, 4 + clo - lo:108 - (hi - chi)] = right[n, :, clo:chi]
        in_mapsA.append({"rawL": slL, "rawR": slR, "rmt": rmv})
    resA = _run(_CACHE["A"], in_mapsA)

    # ---------------- assemble staged canvases for kernel B
    canv = {nm: np.zeros((NH, PITCH), np.float16)
            for nm in ("lcb", "lcr", "rcb", "rcr")}
    for c in range(NC):
        for nm in canv:
            canv[nm][48 * c:48 * (c + 1), :W] = resA.results[c][nm]

    # ---------------- kernel B launch (disparity-sharded)
    in_mapsB = []
    for c in range(NC):
        m = {
            "Lcb": np.ascontiguousarray(canv["lcb"][:, c:c + LW]),
            "Lcr": np.ascontiguousarray(canv["lcr"][:, c:c + LW]),
            "Rcb": np.ascontiguousarray(canv["rcb"][:, :W]),
            "Rcr": np.ascontiguousarray(canv["rcr"][:, :W]),
        }
        in_mapsB.append(m)
    resB = _run(_CACHE["B"], in_mapsB)

    # ---------------- assemble output
    outf = np.empty((N, 3 * D, H, W), np.float32)
    # y-group: de-shear the u8 band scratch (d = u - x) during unshard.
    # 256-byte tail pad keeps the b=4 as_strided view in-bounds.
    flat = np.zeros(NH * 128 * SRW + 256, np.uint8)
    scr = flat[:NH * 128 * SRW].reshape(NH, 128, SRW)
    for c in range(NC):
        scr[48 * c:48 * (c + 1)] = resA.results[c]["scr"]
    s0 = 128 * SRW
    vb = np.lib.stride_tricks.as_strided(
        scr, shape=(D, NH, 4, 128), strides=(1, s0, 256, SRW + 1))
    v4 = np.lib.stride_tricks.as_strided(
        scr[:, :, 1024:], shape=(D, NH, 128), strides=(1, s0, SRW + 1))
    yf = np.empty((D, NH, W), np.float32)
    yf[:, :, 0:512] = vb.reshape(D, NH, 512)
    yf[:, :, 512:W] = v4
    yf -= np.float32(MY)
    yf *= np.float32(1.0 / SY)
    for n in range(N):
        outf[n, 0:D] = yf[:, n * H:(n + 1) * H]
    # u/v groups from kernel B: |.| + bias during unshard
    for c in range(NC):
        o = np.abs(resB.results[c]["out"].astype(np.float32))
        ov = o.reshape(2, NDP, N, H, W)
        for gi, bias in ((0, MU / SU), (1, MV / SV)):
            for dp in range(NDP):
                outf[:, (1 + gi) * D + 8 * dp + c] = ov[gi, dp] - np.float32(bias)
    # constant fills: x >= W-d triangle (all groups) and census border rows (y)
    for gi, cst in ((0, -MY / SY), (1, -MU / SU), (2, -MV / SV)):
        for d in range(1, D):
            outf[:, gi * D + d, :, W - d:] = np.float32(cst)
    outf[:, 0:D, (0, 1, H - 2, H - 1), :] = np.float32(-MY / SY)
    return outf


# revision 17
# speedup vs baseline: 1.3139x; 1.0605x over previous
"""FDSCS front-end as two Bass/Tile kernels on 8 Trainium2 NeuronCores.

Kernel A (row-sharded, 8 cores x 48 half-res rows): avg-pool 2x2 -> YCbCr ->
5x5 census on Y emitted as fp8 bit-planes. The Hamming cost volume runs on the
tensor engine as ONE 24-row fp8 matmul per (row, x-block):
  st = 1-2*Rbit in {-1,+1},  mv = Lbit-0.5*[u<W] in {-0.5,+0.5} (0 for u>=W)
  psum = popL - 2*corr + (popR-12)*[u<W] = ham - 12   (exact integers)
Act/DVE/Pool add +12 and cast to uint8; scr stores the raw 2x band as u8
(1152B contiguous rows). The host de-shears the d = u - x band with a
zero-copy as_strided view and applies (ham-MY)/SY during unshard; border rows
and the x >= W-d triangle are constant host fills (no masks exist on device).

Kernel B (disparity-sharded, cyclic d = 8*dp + core): Cb/Cr cost volumes as
SIGNED f8 diffs (planes pre-scaled by the unify constants in kernel A);
engines rotate over {DVE sub->fp8, DVE sub->f16 + Act cast, Pool sub->fp8}.
The host takes |.|, applies the -M/S bias, and fills the right-edge triangle.
"""

import numpy as np

# ---------------------------------------------------------------- constants
N, HF, WF = 2, 384, 1280       # full-res input (per image): (N, 3, HF, WF)
H, W = 192, 640                # half-res
D = 128                        # disparities
NC = 8                         # cores
RPC = H * N // NC              # 48 half-rows per kernel-A core
UW = 768                       # left (u) extent incl. disparity pad
SRW = 1152                     # scr row width: 4 x 256 + 128 (b=4 trimmed)
PITCH = 768                    # staged plane pitch for kernel B
LW = 760                       # kernel-B left-plane width  (W + 15*8)
NDP = 16                       # disparities per kernel-B core (d = 8*dp + c)
NH = N * H                     # 384 half-rows total
GQ = 12                        # staging row-group quantum

MY, SY = 11.08282948, 0.1949711
MU, SU = 0.02175535, 35.91432953
MV, SV = 0.02679042, 26.79782867

OFFSETS = [(0,0),(1,0),(2,0),(3,0),(4,0),(0,1),(1,1),(2,1),(3,1),(4,1),
           (0,2),(1,2),(3,2),(4,2),(0,3),(1,3),(2,3),(3,3),(4,3),
           (0,4),(1,4),(2,4),(3,4),(4,4)]

# partition layout for kernel A: [2 pad][52 L rows][4 pad][52 R rows][2 pad]
P = 112
LB, RB = 2, 58                 # base partitions of the L / R row blocks

_CACHE = {}


def _bass_mods():
    import concourse.bass as bass
    import concourse.tile as tile
    from concourse import bacc, mybir
    return bass, tile, bacc, mybir


# ================================================================ kernel A
def _build_A():
    bass, tile, bacc, mybir = _bass_mods()
    from concourse._compat import with_exitstack
    from contextlib import ExitStack
    dt = mybir.dt
    Alu = mybir.AluOpType
    ActF = mybir.ActivationFunctionType

    nc = bacc.Bacc("TRN2", target_bir_lowering=False, debug=False, num_devices=NC)
    # 112 full-res rows each: pad rows come in as host zeros so every
    # partition of the raw tile is written (L block: partitions 0:56,
    # R block: 56:112; data at half-rows 2:54 of each block).
    rawL = nc.dram_tensor("rawL", (3, P, WF), dt.float32, kind="ExternalInput").ap()
    rawR = nc.dram_tensor("rawR", (3, P, WF), dt.float32, kind="ExternalInput").ap()
    rmt = nc.dram_tensor("rmt", (P, 2), dt.float32, kind="ExternalInput").ap()
    scr = nc.dram_tensor("scr", (RPC, 128, SRW), dt.uint8, kind="ExternalOutput").ap()
    # staged census planes; ExternalOutput DRAM is pre-zeroed, so stgL's
    # cols [W:UW] read back as zeros (the disparity pad of the mv operand).
    stgL = nc.dram_tensor("stgL", (RPC, 24, UW), dt.float8e4, kind="ExternalOutput").ap()
    stgR = nc.dram_tensor("stgR", (RPC, 24, W), dt.float8e4, kind="ExternalOutput").ap()
    outs = {}
    for nm in ("lcb", "lcr", "rcb", "rcr"):
        outs[nm] = nc.dram_tensor(nm, (RPC, W), dt.float16, kind="ExternalOutput").ap()

    @with_exitstack
    def k(ctx: ExitStack, tc):
        vec, gp, act, sy = nc.vector, nc.gpsimd, nc.scalar, nc.sync
        prep_ctx = ExitStack()
        pool = prep_ctx.enter_context(tc.tile_pool(name="prep", bufs=1))

        raw = pool.tile([P, 3 * 2 * WF], dt.float32, name="raw")
        rv = raw[:].rearrange("p (c j x) -> p c j x", c=3, j=2)
        for ci in range(3):
            for blk, srct in ((0, rawL), (56, rawR)):
                sy.dma_start(rv[blk:blk + 56, ci],
                             srct.rearrange("c (p j) x -> p c j x", j=2)[:, ci])
        rm = pool.tile([P, 2], dt.float32, name="rm")
        sy.dma_start(rm[:], rmt)

        # pooling: horizontal pair sum, vertical pair sum, x0.25 (exact XLA
        # order); h split per channel so it starts on partially-arrived input
        h = pool.tile([P, 3 * 2 * W], dt.float32, name="h")
        hv = h[:].rearrange("p (c j x) -> p c j x", c=3, j=2)
        for ci in range(3):
            vec.tensor_tensor(out=hv[:, ci], in0=rv[:, ci, :, 0::2],
                              in1=rv[:, ci, :, 1::2], op=Alu.add)
        s = pool.tile([P, 3 * W], dt.float32, name="s")
        svw = s[:].rearrange("p (c x) -> p c x", c=3)
        vec.tensor_tensor(out=svw, in0=hv[:, :, 0], in1=hv[:, :, 1], op=Alu.add)
        pooled = pool.tile([P, 3 * W], dt.float32, name="pooled")
        pv = pooled[:].rearrange("p (c x) -> p c x", c=3)
        act.activation(pooled[:], s[:], ActF.Copy, bias=0.0, scale=0.25)

        # Y = (r*.299 + g*.587) + b*.114   (channel order matches XLA assoc)
        t1 = pool.tile([P, W], dt.float32, name="t1")
        act.activation(t1[:], pv[:, 0], ActF.Copy, bias=0.0, scale=0.299)
        t2 = pool.tile([P, W], dt.float32, name="t2")
        vec.tensor_scalar(t2[:], pv[:, 1], 0.587, None, Alu.mult)
        y01 = pool.tile([P, W], dt.float32, name="y01")
        vec.tensor_tensor(out=y01[:], in0=t1[:], in1=t2[:], op=Alu.add)
        t3 = pool.tile([P, W], dt.float32, name="t3")
        act.activation(t3[:], pv[:, 2], ActF.Copy, bias=0.0, scale=0.114)
        Y = pool.tile([P, W], dt.float32, name="Y")
        vec.tensor_tensor(out=Y[:], in0=y01[:], in1=t3[:], op=Alu.add)

        # cb = ((b - y) * 0.564 + 0.5)/SU ; cr = ((r - y) * 0.713 + 0.5)/SV
        cbd = pool.tile([P, W], dt.float32, name="cbd")
        vec.scalar_tensor_tensor(cbd[:], Y[:], -1.0, pv[:, 2], Alu.mult, Alu.add)
        cb16 = pool.tile([P, W], dt.float16, name="cb16")
        act.activation(cb16[:], cbd[:], ActF.Copy, bias=0.5 / SU, scale=0.564 / SU)
        crd = pool.tile([P, W], dt.float32, name="crd")
        vec.scalar_tensor_tensor(crd[:], Y[:], -1.0, pv[:, 0], Alu.mult, Alu.add)
        cr16 = pool.tile([P, W], dt.float16, name="cr16")
        act.activation(cr16[:], crd[:], ActF.Copy, bias=0.5 / SV, scale=0.713 / SV)
        for nm, t, blk in [("lcb", cb16, LB), ("lcr", cr16, LB),
                           ("rcb", cb16, RB), ("rcr", cr16, RB)]:
            sy.dma_start(outs[nm], t[blk + 2:blk + 50, :])

        # Y's pad rows are computed zeros (host-zero raw pads), so the +-2
        # partition shifts pull zeros across image boundaries for free.
        # partition-shifted copies of Y: one whole-tile DMA per row offset
        ysh = {0: Y}
        for dv in (-2, -1, 1, 2):
            t = pool.tile([P, W], dt.float32, name=f"ysh{dv + 2}")
            if dv > 0:
                sy.dma_start(t[0:P - dv], Y[dv:P])
                sy.dma_start(t[P - dv:P], Y[0:dv])   # filler: rows unused
            else:
                sy.dma_start(t[-dv:P], Y[0:P + dv])
                sy.dma_start(t[0:-dv], Y[0:-dv])     # filler: rows unused
            ysh[dv] = t

        # census bits as fp8 planes [P, 24, W]; 2px x-borders stay zero.
        # engine rotation: DVE 663ns / Pool 1261ns per op — Pool takes a share
        # so DVE keeps room for the affine + its norm chunks.
        WI = W - 4
        bits = pool.tile([P, 24 * W], dt.float8e4, name="bits")
        bv = bits[:].rearrange("p (k x) -> p k x", k=24)
        vec.memset(bv[:, :, 0:2], 0.0)
        vec.memset(bv[:, :, W - 2:W], 0.0)
        for k_i, (u, v) in enumerate(OFFSETS):
            src = ysh[v - 2]
            eng = vec
            eng.tensor_tensor(out=bv[:, k_i, 2:W - 2],
                              in0=src[:, u:u + WI],
                              in1=Y[:, 2:W - 2], op=Alu.is_ge)
        # one per-partition affine pass builds both staged operand forms:
        #   L rows: bit - 0.5          R rows: 1 - 2*bit
        bk = bits[:].rearrange("p (k x) -> p k x", k=24)
        vec.tensor_scalar(bk[:, 0:18], bk[:, 0:18], rm[:, 0:1], rm[:, 1:2],
                          Alu.mult, Alu.add)
        gp.tensor_scalar(bk[:, 18:24], bk[:, 18:24], rm[:, 0:1], rm[:, 1:2],
                          Alu.mult, Alu.add)
        for g0 in range(0, RPC, GQ):
            sy.dma_start(stgL[g0:g0 + GQ, :, 0:W], bv[LB + 2 + g0:LB + 2 + g0 + GQ])
            sy.dma_start(stgR[g0:g0 + GQ], bv[RB + 2 + g0:RB + 2 + g0 + GQ])
        prep_ctx.close()  # free all prep tiles before the plane pool opens

        # matmul operand planes, k-major: st [24, q, W], mv [24, q, UW];
        # one tile pair per GQ-row group so early groups' matmuls don't wait
        # on later groups' loads. Loads ride the Act queue (SP has the
        # staging stores).
        mplane = ctx.enter_context(tc.tile_pool(name="mplane", bufs=1))
        stg_tiles = []
        for g0 in range(0, RPC, GQ):
            st = mplane.tile([24, GQ * W], dt.float8e4, name=f"stp{g0}")
            mv = mplane.tile([24, GQ * UW], dt.float8e4, name=f"mvp{g0}")
            sl = bass.AP(stgL.tensor, g0 * 24 * UW, [[UW, 24], [24 * UW, GQ], [1, UW]])
            sr = bass.AP(stgR.tensor, g0 * 24 * W, [[W, 24], [24 * W, GQ], [1, W]])
            act.dma_start(mv[:].rearrange("p (q u) -> p q u", q=GQ), sl)
            act.dma_start(st[:].rearrange("p (q x) -> p q x", q=GQ), sr)
            stg_tiles.append((st, mv))

        ppool = ctx.enter_context(tc.tile_pool(name="ps", bufs=2, space="PSUM"))
        pcpool = ctx.enter_context(tc.tile_pool(name="psc", bufs=2, space="PSUM"))
        opool = ctx.enter_context(tc.tile_pool(name="ob", bufs=3))
        RB4 = 4  # rows per scr store
        for r0 in range(0, RPC, RB4):
            o = opool.tile([128, RB4 * SRW], dt.uint8, name="o")
            for ri in range(RB4):
                r = r0 + ri
                stq, mvq = stg_tiles[r // GQ]
                stv = stq[:].rearrange("p (q x) -> p q x", q=GQ)
                mvv = mvq[:].rearrange("p (q u) -> p q u", q=GQ)
                rq = r % GQ
                psA = ppool.tile([128, 512], dt.float32, name="psA")
                psB = ppool.tile([128, 512], dt.float32, name="psB")
                psC = pcpool.tile([128, 128], dt.float32, name="psC")
                for b in range(4):
                    ps = psA if b < 2 else psB
                    nc.tensor.matmul(
                        ps[:, (b % 2) * 256:(b % 2) * 256 + 256],
                        stv[:, rq, b * 128:(b + 1) * 128],
                        mvv[:, rq, b * 128:b * 128 + 256],
                        start=True, stop=True,
                    )
                nc.tensor.matmul(psC[:], stv[:, rq, 512:640], mvv[:, rq, 512:640],
                                 start=True, stop=True)
                oo = ri * SRW
                # +12 and cast to u8; c0 on Act, c1 on DVE, c2 alternating
                e2 = act if r % 2 == 0 else vec
                chunks = [(o[:, oo:oo + 512], psA, act),
                          (o[:, oo + 512:oo + 1024], psB, vec),
                          (o[:, oo + 1024:oo + 1152], psC, e2)]
                for dst, ps, eng in chunks:
                    if eng is act:
                        act.activation(dst, ps[:], ActF.Copy, bias=12.0, scale=1.0)
                    else:
                        eng.tensor_scalar(dst, ps[:], 1.0, 12.0, Alu.mult, Alu.add)
            gp.dma_start(scr[r0:r0 + RB4].rearrange("r p u -> p r u"),
                         o[:].rearrange("p (r u) -> p r u", r=RB4))

    with tile.TileContext(nc) as tc:
        k(tc)
    nc.compile()
    return nc


# ================================================================ kernel B
def _build_B():
    bass, tile, bacc, mybir = _bass_mods()
    from concourse._compat import with_exitstack
    from contextlib import ExitStack
    dt = mybir.dt
    Alu = mybir.AluOpType
    ActF = mybir.ActivationFunctionType

    nc = bacc.Bacc("TRN2", target_bir_lowering=False, debug=False, num_devices=NC)
    ins = {}
    for nm, wid in [("Lcb", LW), ("Lcr", LW), ("Rcb", W), ("Rcr", W)]:
        ins[nm] = nc.dram_tensor(nm, (NH, wid), dt.float16, kind="ExternalInput").ap()
    out = nc.dram_tensor("out", (2, NDP, NH, W), dt.float8e4, kind="ExternalOutput").ap()

    RG = NH // 128  # 3 row groups

    @with_exitstack
    def k(ctx: ExitStack, tc):
        vec, gp, act, sy = nc.vector, nc.gpsimd, nc.scalar, nc.sync

        plane_pool = ctx.enter_context(tc.tile_pool(name="planes", bufs=1))
        planes = {}
        for nm in ("Lcb", "Lcr", "Rcb", "Rcr"):
            wid = LW if nm.startswith("L") else W
            t = plane_pool.tile([128, RG * wid], dt.float16, name=f"pl_{nm}")
            sy.dma_start(t[:].rearrange("p (g x) -> p g x", g=RG),
                         ins[nm].rearrange("(g p) x -> p g x", p=128))
            planes[nm] = t

        dpool = ctx.enter_context(tc.tile_pool(name="dp", bufs=4))
        fpool = ctx.enter_context(tc.tile_pool(name="fp", bufs=4))

        def Lv(nm, off, wt):
            return planes[nm][:].rearrange("p (g x) -> p g x", g=RG)[:, :, off:off + wt]

        def Rv(nm, wt):
            return planes[nm][:].rearrange("p (g x) -> p g x", g=RG)[:, :, 0:wt]

        # signed diffs only; |.|, bias, and the right-edge triangle are host
        # work during unshard. x >= W-8*dp is never computed (host constant).
        # engine rotation P,A,A,D: Pool-direct 8, DVE-sub+Act-cast 16,
        # DVE-direct 8.
        PAT = ("A", "A", "D", "P")
        for dp in range(NDP):
            wt = W - 8 * dp
            for gi, lnm, rnm in ((0, "Lcb", "Rcb"), (1, "Lcr", "Rcr")):
                i = 2 * dp + gi
                kind = PAT[i % 4]
                c8 = fpool.tile([128, RG * W], dt.float8e4, name="c8")
                c8v = c8[:].rearrange("p (g x) -> p g x", g=RG)[:, :, 0:wt]
                if kind == "P":
                    gp.tensor_tensor(out=c8v, in0=Lv(lnm, 8 * dp, wt),
                                     in1=Rv(rnm, wt), op=Alu.subtract)
                    steng = gp
                elif kind == "D":
                    vec.tensor_tensor(out=c8v, in0=Lv(lnm, 8 * dp, wt),
                                      in1=Rv(rnm, wt), op=Alu.subtract)
                    steng = sy
                else:
                    du = dpool.tile([128, RG * W], dt.float16, name="du")
                    duv = du[:].rearrange("p (g x) -> p g x", g=RG)[:, :, 0:wt]
                    vec.tensor_tensor(out=duv, in0=Lv(lnm, 8 * dp, wt),
                                      in1=Rv(rnm, wt), op=Alu.subtract)
                    act.activation(c8v, duv, ActF.Copy, bias=0.0, scale=1.0)
                    steng = act
                steng.dma_start(
                    out[gi, dp].rearrange("(g p) x -> p g x", p=128)[:, :, 0:wt],
                    c8v)

    with tile.TileContext(nc) as tc:
        k(tc)
    nc.compile()
    return nc


# ================================================================ host
def _run(nc, in_maps):
    from concourse.bass_utils import run_bass_kernel_spmd
    return run_bass_kernel_spmd(nc, in_maps, core_ids=list(range(NC)))


def kernel(left, right):
    left = np.asarray(left, dtype=np.float32)
    right = np.asarray(right, dtype=np.float32)

    if "A" not in _CACHE:
        _CACHE["A"] = _build_A()
    if "B" not in _CACHE:
        _CACHE["B"] = _build_B()

    # ---------------- kernel A launch (row-sharded)
    rmv = np.zeros((P, 2), np.float32)
    rmv[LB:LB + 52] = (1.0, -0.5)      # L rows: bit - 0.5
    rmv[RB:RB + 52] = (-2.0, 1.0)      # R rows: 1 - 2*bit
    in_mapsA = []
    for c in range(NC):
        n, r0 = c // 4, 48 * (c % 4)
        lo, hi = 2 * r0 - 4, 2 * (r0 + RPC) + 4
        slL = np.zeros((3, P, WF), np.float32)
        slR = np.zeros((3, P, WF), np.float32)
        clo, chi = max(lo, 0), min(hi, HF)
        # data occupies full-res rows [4:108] (pad rows 0:4 / 108:112 stay 0)
        slL[:, 4 + clo - lo:108 - (hi - chi)] = left[n, :, clo:chi]
        slR[:, 4 + clo - lo:108 - (hi - chi)] = right[n, :, clo:chi]
        in_mapsA.append({"rawL": slL, "rawR": slR, "rmt": rmv})
    resA = _run(_CACHE["A"], in_mapsA)

    # ---------------- assemble staged canvases for kernel B
    canv = {nm: np.zeros((NH, PITCH), np.float16)
            for nm in ("lcb", "lcr", "rcb", "rcr")}
    for c in range(NC):
        for nm in canv:
            canv[nm][48 * c:48 * (c + 1), :W] = resA.results[c][nm]

    # ---------------- kernel B launch (disparity-sharded)
    in_mapsB = []
    for c in range(NC):
        m = {
            "Lcb": np.ascontiguousarray(canv["lcb"][:, c:c + LW]),
            "Lcr": np.ascontiguousarray(canv["lcr"][:, c:c + LW]),
            "Rcb": np.ascontiguousarray(canv["rcb"][:, :W]),
            "Rcr": np.ascontiguousarray(canv["rcr"][:, :W]),
        }
        in_mapsB.append(m)
    resB = _run(_CACHE["B"], in_mapsB)

    # ---------------- assemble output
    outf = np.empty((N, 3 * D, H, W), np.float32)
    # y-group: de-shear the u8 band scratch (d = u - x) during unshard.
    # 256-byte tail pad keeps the b=4 as_strided view in-bounds.
    flat = np.zeros(NH * 128 * SRW + 256, np.uint8)
    scr = flat[:NH * 128 * SRW].reshape(NH, 128, SRW)
    for c in range(NC):
        scr[48 * c:48 * (c + 1)] = resA.results[c]["scr"]
    s0 = 128 * SRW
    vb = np.lib.stride_tricks.as_strided(
        scr, shape=(D, NH, 4, 128), strides=(1, s0, 256, SRW + 1))
    v4 = np.lib.stride_tricks.as_strided(
        scr[:, :, 1024:], shape=(D, NH, 128), strides=(1, s0, SRW + 1))
    yf = np.empty((D, NH, W), np.float32)
    yf[:, :, 0:512] = vb.reshape(D, NH, 512)
    yf[:, :, 512:W] = v4
    yf -= np.float32(MY)
    yf *= np.float32(1.0 / SY)
    for n in range(N):
        outf[n, 0:D] = yf[:, n * H:(n + 1) * H]
    # u/v groups from kernel B: |.| + bias during unshard
    for c in range(NC):
        o = np.abs(resB.results[c]["out"].astype(np.float32))
        ov = o.reshape(2, NDP, N, H, W)
        for gi, bias in ((0, MU / SU), (1, MV / SV)):
            for dp in range(NDP):
                outf[:, (1 + gi) * D + 8 * dp + c] = ov[gi, dp] - np.float32(bias)
    # constant fills: x >= W-d triangle (all groups) and census border rows (y)
    for gi, cst in ((0, -MY / SY), (1, -MU / SU), (2, -MV / SV)):
        for d in range(1, D):
            outf[:, gi * D + d, :, W - d:] = np.float32(cst)
    outf[:, 0:D, (0, 1, H - 2, H - 1), :] = np.float32(-MY / SY)
    return outf
